# revision 4
# baseline (speedup 1.0000x reference)
"""HGNN forward kernel for Trainium2, 8 NeuronCores, data-parallel over batch.

Device program (per core, batch chunk of 128) — unchanged from the working
baseline:
  - Embedding-row gathers via gpsimd indirect_dma_start, 128 rows/instruction
    (one row per partition, offsets [128,1] int32 in SBUF).
  - Neighbor-group sums computed by DMA-side accumulation (compute_op=add).
  - Math algebraically folded so every matmul is a 64x64 weight applied to
    transposed activations [64, N]; mean-over-neighbors pushed through the
    linear maps; avg_real weights computed on-device from the raw indices.

Execution path: custom PJRT shard_map executor (modeled on
bass2jax.run_bass_via_pjrt) with committed-device-array input caching.
Under axon, shipping the 8x-replicated embedding tables (~107MB) through
the tunnel dominates wall time; here every input is device_put once and
re-used on subsequent calls when its host content is unchanged (verified
by np.array_equal against a stored copy), so a steady-state call uploads
only the donated output buffer and runs the NEFF.
"""
import numpy as np

import concourse.bass as bass
import concourse.bacc as bacc
import concourse.mybir as mybir
import concourse.tile as tile
from concourse.masks import make_identity

F32 = mybir.dt.float32
I32 = mybir.dt.int32
AF = mybir.ActivationFunctionType
OP = mybir.AluOpType

NUM_SYMP, NUM_DISE = 50000, 2000
D = 64
B = 1024
NCORES = 8
BC = B // NCORES  # 128 batch elems per core

_CACHE = {}
_LAST_EXEC_NS = None


def _bcast_inner(ap, n):
    """Append a broadcast (step-0) innermost dim of size n to an AP."""
    return bass.AP(ap.tensor, ap.offset, list(ap.ap) + [[0, n]])


def _bcast_mid(ap, pos, n):
    """Insert a broadcast (step-0) dim of size n at position pos."""
    dims = list(ap.ap)
    return bass.AP(ap.tensor, ap.offset, dims[:pos] + [[0, n]] + dims[pos:])


def _build():
    nc = bacc.Bacc("TRN2", target_bir_lowering=False, debug=False)

    Es = nc.dram_tensor("Es", [NUM_SYMP + 1, D], F32, kind="ExternalInput")
    Ed = nc.dram_tensor("Ed", [NUM_DISE + 1, D], F32, kind="ExternalInput")
    wn = ["w_dsd_21", "w_dsd_22", "w_dsd_11", "w_dsd_12",
          "w_usu_3", "w_usu_21", "w_usu_22", "w_usu_1"]
    W = {n: nc.dram_tensor(n, [D, D], F32, kind="ExternalInput") for n in wn}
    i_label = nc.dram_tensor("i_label", [BC, 1], I32, kind="ExternalInput")
    i_dsd1 = nc.dram_tensor("i_dsd1", [BC, 8], I32, kind="ExternalInput")
    i_dsd2 = nc.dram_tensor("i_dsd2", [BC, 64], I32, kind="ExternalInput")
    i_usu1 = nc.dram_tensor("i_usu1", [BC, 8], I32, kind="ExternalInput")
    i_usu2 = nc.dram_tensor("i_usu2", [BC, 64], I32, kind="ExternalInput")
    i_usu3 = nc.dram_tensor("i_usu3", [BC, 1024], I32, kind="ExternalInput")
    out = nc.dram_tensor("score", [1, BC], F32, kind="ExternalOutput")

    with tile.TileContext(nc) as tc:
        with tc.tile_pool(name="const", bufs=1) as cst, \
             tc.tile_pool(name="main", bufs=1) as mp, \
             tc.tile_pool(name="ps", bufs=4, space="PSUM") as ps, \
             tc.tile_pool(name="psm", bufs=3, space="PSUM") as psm:

            ident = cst.tile([128, 128], F32)
            make_identity(nc, ident[:])
            ones1 = cst.tile([1, D], F32)
            nc.vector.memset(ones1[:], 1.0)
            ones64 = cst.tile([D, 1], F32)
            nc.vector.memset(ones64[:], 1.0)
            wt = {}
            for n in wn:
                wt[n] = cst.tile([D, D], F32, name=f"wt_{n}")
                nc.sync.dma_start(out=wt[n][:], in_=W[n][:])

            # ---- index tiles (single DMAs) ----
            ix_lab = mp.tile([BC, 1], I32)
            nc.sync.dma_start(out=ix_lab[:], in_=i_label[:])
            ix_d1 = mp.tile([BC, 8], I32)
            nc.sync.dma_start(out=ix_d1[:], in_=i_dsd1[:])
            ix_d2 = mp.tile([BC, 64], I32)
            nc.sync.dma_start(out=ix_d2[:], in_=i_dsd2[:])
            ix_u1 = mp.tile([BC, 8], I32)
            nc.sync.dma_start(out=ix_u1[:], in_=i_usu1[:])
            ix_u2 = mp.tile([BC, 64], I32)
            nc.sync.dma_start(out=ix_u2[:], in_=i_usu2[:])
            ix_u3 = mp.tile([BC, 1024], I32)
            nc.sync.dma_start(out=ix_u3[:], in_=i_usu3[:])

            def gather(dst_ap, table, off_ap, accum=False):
                nc.gpsimd.indirect_dma_start(
                    out=dst_ap, out_offset=None, in_=table[:],
                    in_offset=bass.IndirectOffsetOnAxis(ap=off_ap, axis=0),
                    compute_op=(OP.add if accum else OP.bypass),
                )


            def lrelu(dst_ap, src_ap, scratch_name):
                t = mp.tile(list(dst_ap.shape), F32, name=scratch_name, tag="lrt")
                nc.vector.tensor_scalar_mul(out=t[:], in0=src_ap, scalar1=0.2)
                nc.vector.tensor_tensor(out=dst_ap, in0=src_ap, in1=t[:], op=OP.max)

            # ---- plain gathers: td, es, u1 (rows used individually) ----
            td_std = mp.tile([BC, D], F32)
            gather(td_std[:], Ed, ix_lab[:, 0:1])
            es_std = mp.tile([BC, 8 * D], F32)
            u1_std = mp.tile([BC, 8 * D], F32)
            for h in range(8):
                gather(es_std[:, h * D:(h + 1) * D], Es, ix_d1[:, h:h + 1])
                gather(u1_std[:, h * D:(h + 1) * D], Es, ix_u1[:, h:h + 1])

            # ---- accumulating gathers: dsd_2 (8 nbrs), usu_3 (16 nbrs) ----
            acc_d2 = mp.tile([BC, 8 * D], F32)
            nc.vector.memset(acc_d2[:], 0.0)
            acc_u3 = mp.tile([BC, 64 * D], F32)
            nc.vector.memset(acc_u3[:], 0.0)
            for j in range(8):
                for m in range(8):
                    gather(acc_d2[:, m * D:(m + 1) * D], Ed,
                           ix_d2[:, m * 8 + j: m * 8 + j + 1], accum=True)
            for j in range(16):
                for m in range(64):
                    gather(acc_u3[:, m * D:(m + 1) * D], Es,
                           ix_u3[:, m * 16 + j: m * 16 + j + 1], accum=True)

            # ---- count weights w = (cnt>0) / (cnt + 1e-8) ----
            def count_w(ix_t, groups, j, name):
                f = mp.tile([BC, groups * j], F32, name=f"f_{name}")
                nc.vector.tensor_copy(out=f[:], in_=ix_t[:])
                z = mp.tile([BC, groups * j], F32, name=f"z_{name}")
                nc.vector.tensor_scalar(out=z[:], in0=f[:], scalar1=0.0,
                                        scalar2=None, op0=OP.is_equal)
                zc = mp.tile([BC, groups], F32, name=f"zc_{name}")
                nc.vector.tensor_reduce(
                    out=zc[:],
                    in_=z[:].rearrange("p (g j) -> p g j", g=groups, j=j),
                    axis=mybir.AxisListType.X, op=OP.add)
                cnt = mp.tile([BC, groups], F32, name=f"cnt_{name}")
                nc.vector.tensor_scalar(out=cnt[:], in0=zc[:], scalar1=-1.0,
                                        scalar2=float(j), op0=OP.mult, op1=OP.add)
                mpos = mp.tile([BC, groups], F32, name=f"mp_{name}")
                nc.vector.tensor_scalar(out=mpos[:], in0=cnt[:], scalar1=1.0,
                                        scalar2=None, op0=OP.min)
                ce = mp.tile([BC, groups], F32, name=f"ce_{name}")
                nc.vector.tensor_scalar(out=ce[:], in0=cnt[:], scalar1=1e-8,
                                        scalar2=None, op0=OP.add)
                r = mp.tile([BC, groups], F32, name=f"r_{name}")
                nc.vector.reciprocal(out=r[:], in_=ce[:])
                w = mp.tile([BC, groups], F32, name=f"w_{name}")
                nc.vector.tensor_tensor(out=w[:], in0=r[:], in1=mpos[:], op=OP.mult)
                return w

            w_d2 = count_w(ix_d2, 8, 8, "d2")     # [128, 8]
            w_u3 = count_w(ix_u3, 64, 16, "u3")   # [128, 64]
            w_u2 = count_w(ix_u2, 8, 8, "u2")     # [128, 8]
            w_d1 = count_w(ix_d1, 1, 8, "d1")     # [128, 1]
            w_u1 = count_w(ix_u1, 1, 8, "u1")     # [128, 1]

            # ---- scale accumulated sums by group weights (std layout) ----
            nc.vector.tensor_tensor(
                out=acc_d2[:].rearrange("p (m d) -> p m d", m=8, d=D),
                in0=acc_d2[:].rearrange("p (m d) -> p m d", m=8, d=D),
                in1=_bcast_inner(w_d2[:], D), op=OP.mult)
            nc.vector.tensor_tensor(
                out=acc_u3[:].rearrange("p (m d) -> p m d", m=64, d=D),
                in0=acc_u3[:].rearrange("p (m d) -> p m d", m=64, d=D),
                in1=_bcast_inner(w_u3[:], D), op=OP.mult)

            # ---- transposes into [64, cols] matmul layout ----
            def transpose_into(dstT, src_std, nblk):
                for m in range(nblk):
                    p = ps.tile([D, 128], F32, name="tp", tag="tp")
                    nc.tensor.transpose(out=p[:], in_=src_std[:, m * D:(m + 1) * D],
                                        identity=ident[:])
                    nc.vector.tensor_copy(out=dstT[:, m * 128:(m + 1) * 128], in_=p[:])

            tdT = mp.tile([D, 128], F32)
            transpose_into(tdT, td_std, 1)
            esT = mp.tile([D, 8 * 128], F32)
            transpose_into(esT, es_std, 8)
            u1T = mp.tile([D, 8 * 128], F32)
            transpose_into(u1T, u1_std, 8)
            edmT = mp.tile([D, 8 * 128], F32)
            transpose_into(edmT, acc_d2, 8)
            s3T = mp.tile([D, 64 * 128], F32)
            transpose_into(s3T, acc_u3, 64)

            # ---- replicated column weights via transpose + K=1 matmul ----
            def replicate_cols(w_t, groups, name):
                rep = mp.tile([D, groups * 128], F32, name=f"rep_{name}")
                for g in range(groups):
                    pt = ps.tile([2, 128], F32, name="wtp", tag="tp")
                    nc.tensor.transpose(out=pt[0:1, :], in_=w_t[:, g:g + 1],
                                        identity=ident[:])
                    wg = mp.tile([1, 128], F32, name=f"wg_{name}")
                    nc.vector.tensor_copy(out=wg[:], in_=pt[0:1, :])
                    pr = ps.tile([D, 128], F32, name="wrep", tag="tp")
                    nc.tensor.matmul(out=pr[:], lhsT=ones1[:], rhs=wg[:],
                                     start=True, stop=True)
                    nc.vector.tensor_copy(out=rep[:, g * 128:(g + 1) * 128], in_=pr[:])
                return rep

            w2u_rep = replicate_cols(w_u2, 8, "u2")    # [64, 1024]
            w1u_rep = replicate_cols(w_u1, 1, "u1")    # [64, 128]
            w1d_rep = replicate_cols(w_d1, 1, "d1")    # [64, 128]

            # ---- usu path ----
            # eu2 = lrelu(W3 @ (w3 * sum_j s3)) ; cols (m=u1*8+u2, b)
            eu2T = mp.tile([D, 64 * 128], F32)
            for ch in range(16):
                pm = psm.tile([D, 512], F32, name="mm3", tag="mm")
                nc.tensor.matmul(out=pm[:], lhsT=wt["w_usu_3"][:],
                                 rhs=s3T[:, ch * 512:(ch + 1) * 512],
                                 start=True, stop=True)
                lrelu(eu2T[:, ch * 512:(ch + 1) * 512], pm[:], "lr3")

            # su1 = sum_u2 eu2 ; su2 = sum_u2 (eu2 * u1)  -> cols (u1, b)
            su1 = mp.tile([D, 8 * 128], F32)
            ev = eu2T[:].rearrange("p (u v b) -> p u b v", u=8, v=8, b=128)
            nc.vector.tensor_reduce(
                out=su1[:].rearrange("p (u b) -> p u b", u=8, b=128),
                in_=ev, axis=mybir.AxisListType.X, op=OP.add)
            tmp = mp.tile([D, 64 * 128], F32)
            u1bc = _bcast_mid(u1T[:].rearrange("p (u b) -> p u b", u=8, b=128), 2, 8)
            nc.vector.tensor_tensor(
                out=tmp[:].rearrange("p (u v b) -> p u v b", u=8, v=8, b=128),
                in0=eu2T[:].rearrange("p (u v b) -> p u v b", u=8, v=8, b=128),
                in1=u1bc, op=OP.mult)
            su2 = mp.tile([D, 8 * 128], F32)
            nc.vector.tensor_reduce(
                out=su2[:].rearrange("p (u b) -> p u b", u=8, b=128),
                in_=tmp[:].rearrange("p (u v b) -> p u b v", u=8, v=8, b=128),
                axis=mybir.AxisListType.X, op=OP.add)

            # rhs1 = su1*w2 + u1T ; rhs2 = su2*w2
            rhs1 = mp.tile([D, 8 * 128], F32)
            nc.vector.tensor_tensor(out=rhs1[:], in0=su1[:], in1=w2u_rep[:], op=OP.mult)
            nc.vector.tensor_tensor(out=rhs1[:], in0=rhs1[:], in1=u1T[:], op=OP.add)
            rhs2 = mp.tile([D, 8 * 128], F32)
            nc.vector.tensor_tensor(out=rhs2[:], in0=su2[:], in1=w2u_rep[:], op=OP.mult)

            es1 = mp.tile([D, 8 * 128], F32)
            for ch in range(2):
                sl = slice(ch * 512, (ch + 1) * 512)
                pm = psm.tile([D, 512], F32, name="mmu", tag="mm")
                nc.tensor.matmul(out=pm[:], lhsT=wt["w_usu_21"][:], rhs=rhs1[:, sl],
                                 start=True, stop=False)
                nc.tensor.matmul(out=pm[:], lhsT=wt["w_usu_22"][:], rhs=rhs2[:, sl],
                                 start=False, stop=True)
                lrelu(es1[:, sl], pm[:], "lru")

            # emb_user = lrelu(W1u @ (w1u * sum_u1 es1))
            rU = mp.tile([D, 128], F32)
            nc.vector.tensor_reduce(
                out=rU[:],
                in_=es1[:].rearrange("p (u b) -> p b u", u=8, b=128),
                axis=mybir.AxisListType.X, op=OP.add)
            nc.vector.tensor_tensor(out=rU[:], in0=rU[:], in1=w1u_rep[:], op=OP.mult)
            pmU = ps.tile([D, 128], F32, name="mmU", tag="tp")
            nc.tensor.matmul(out=pmU[:], lhsT=wt["w_usu_1"][:], rhs=rU[:],
                             start=True, stop=True)
            embU = mp.tile([D, 128], F32)
            lrelu(embU[:], pmU[:], "lrU")

            # ---- dsd path ----
            rhsA = mp.tile([D, 8 * 128], F32)
            nc.vector.tensor_tensor(out=rhsA[:], in0=edmT[:], in1=esT[:], op=OP.add)
            rhsB = mp.tile([D, 8 * 128], F32)
            nc.vector.tensor_tensor(out=rhsB[:], in0=edmT[:], in1=esT[:], op=OP.mult)
            es1d = mp.tile([D, 8 * 128], F32)
            for ch in range(2):
                sl = slice(ch * 512, (ch + 1) * 512)
                pm = psm.tile([D, 512], F32, name="mmd", tag="mm")
                nc.tensor.matmul(out=pm[:], lhsT=wt["w_dsd_21"][:], rhs=rhsA[:, sl],
                                 start=True, stop=False)
                nc.tensor.matmul(out=pm[:], lhsT=wt["w_dsd_22"][:], rhs=rhsB[:, sl],
                                 start=False, stop=True)
                lrelu(es1d[:, sl], pm[:], "lrd")

            r1 = mp.tile([D, 128], F32)
            nc.vector.tensor_reduce(
                out=r1[:],
                in_=es1d[:].rearrange("p (h b) -> p b h", h=8, b=128),
                axis=mybir.AxisListType.X, op=OP.add)
            tmp2 = mp.tile([D, 8 * 128], F32)
            tdbc = _bcast_mid(tdT[:], 1, 8)
            nc.vector.tensor_tensor(
                out=tmp2[:].rearrange("p (h b) -> p h b", h=8, b=128),
                in0=es1d[:].rearrange("p (h b) -> p h b", h=8, b=128),
                in1=tdbc, op=OP.mult)
            r2 = mp.tile([D, 128], F32)
            nc.vector.tensor_reduce(
                out=r2[:],
                in_=tmp2[:].rearrange("p (h b) -> p b h", h=8, b=128),
                axis=mybir.AxisListType.X, op=OP.add)
            m1 = mp.tile([D, 128], F32)
            nc.vector.tensor_tensor(out=m1[:], in0=r1[:], in1=w1d_rep[:], op=OP.mult)
            nc.vector.tensor_tensor(out=m1[:], in0=m1[:], in1=tdT[:], op=OP.add)
            m2 = mp.tile([D, 128], F32)
            nc.vector.tensor_tensor(out=m2[:], in0=r2[:], in1=w1d_rep[:], op=OP.mult)
            pmD = ps.tile([D, 128], F32, name="mmD", tag="tp")
            nc.tensor.matmul(out=pmD[:], lhsT=wt["w_dsd_11"][:], rhs=m1[:],
                             start=True, stop=False)
            nc.tensor.matmul(out=pmD[:], lhsT=wt["w_dsd_12"][:], rhs=m2[:],
                             start=False, stop=True)
            embD = mp.tile([D, 128], F32)
            lrelu(embD[:], pmD[:], "lrD")

            # ---- score ----
            prod = mp.tile([D, 128], F32)
            nc.vector.tensor_tensor(out=prod[:], in0=embD[:], in1=embU[:], op=OP.mult)
            pS = ps.tile([2, 128], F32, name="mmS", tag="tp")
            nc.tensor.matmul(out=pS[0:1, :], lhsT=ones64[:], rhs=prod[:],
                             start=True, stop=True)
            score_sb = mp.tile([1, 128], F32)
            nc.vector.tensor_copy(out=score_sb[:], in_=pS[0:1, :])
            nc.sync.dma_start(out=out[:], in_=score_sb[:])

    nc.finalize()
    return nc


class _Executor:
    """shard_map/PJRT executor with committed-device-array input caching.

    Per BIR input we keep (host_copy, committed_global_array). On each call
    the freshly prepped host value is compared against host_copy; on match
    the committed jax.Array (already resident on the 8 cores with the right
    sharding) is passed to jit directly, so no bytes cross the axon tunnel.
    """

    def __init__(self):
        import jax
        from jax.sharding import Mesh, PartitionSpec, NamedSharding
        from jax.experimental.shard_map import shard_map
        from concourse import bass2jax

        self.jax = jax
        bass2jax.install_neuronx_cc_hook()
        nc = _build()
        assert nc.dbg_addr is None
        self.nc = nc
        partition_name = (nc.partition_id_tensor.name
                          if nc.partition_id_tensor else None)

        in_names, out_names, out_avals, zero_shapes = [], [], [], []
        for alloc in nc.m.functions[0].allocations:
            if not isinstance(alloc, mybir.MemoryLocationSet):
                continue
            assert alloc.memorylocations
            name = alloc.memorylocations[0].name
            if alloc.kind == "ExternalInput":
                if name != partition_name:
                    in_names.append(name)
            elif alloc.kind == "ExternalOutput":
                shape = tuple(alloc.tensor_shape)
                dtype = mybir.dt.np(alloc.dtype)
                out_names.append(name)
                out_avals.append(jax.core.ShapedArray(shape, dtype))
                zero_shapes.append((shape, dtype))
        self.in_names = in_names
        self.out_names = out_names
        self.out_avals = out_avals
        self.zero_shapes = zero_shapes
        n_params = len(in_names)
        n_outs = len(out_names)

        devices = jax.devices()[:NCORES]
        assert len(devices) == NCORES
        self.devices = devices
        self.mesh = Mesh(np.asarray(devices), ("core",))
        self.sharding = NamedSharding(self.mesh, PartitionSpec("core"))

        all_names = tuple(in_names) + tuple(out_names)
        if partition_name is not None:
            all_names = all_names + (partition_name,)

        def _body(*args):
            operands = list(args)
            if partition_name is not None:
                operands.append(bass2jax.partition_id_tensor())
            outs = bass2jax._bass_exec_p.bind(
                *operands,
                out_avals=tuple(out_avals),
                in_names=all_names,
                out_names=tuple(out_names),
                lowering_input_output_aliases=(),
                sim_require_finite=True,
                sim_require_nnan=True,
                nc=nc,
            )
            return tuple(outs)

        donate = tuple(range(n_params, n_params + n_outs))
        self.fn = jax.jit(
            shard_map(_body, mesh=self.mesh,
                      in_specs=(PartitionSpec("core"),) * (n_params + n_outs),
                      out_specs=(PartitionSpec("core"),) * n_outs,
                      check_rep=False),
            donate_argnums=donate, keep_unused=True)

        # name -> (host_copy, committed jax.Array)
        self._committed = {}

    def _put(self, name, shards):
        """Commit per-core host shards as one global array on the 8 cores."""
        jax = self.jax
        dev_arrs = [jax.device_put(a, d) for a, d in zip(shards, self.devices)]
        gshape = (NCORES * shards[0].shape[0],) + tuple(shards[0].shape[1:])
        return jax.make_array_from_single_device_arrays(
            gshape, self.sharding, dev_arrs)

    def feed(self, name, host_val, make_shards):
        """Return committed array for `name`, re-uploading iff content changed.

        host_val: cheap-to-compare host array identifying the content.
        make_shards: () -> list of NCORES per-core np arrays (called lazily,
        only on miss).
        """
        ent = self._committed.get(name)
        if ent is not None and host_val.shape == ent[0].shape \
                and host_val.dtype == ent[0].dtype \
                and np.array_equal(host_val, ent[0]):
            return ent[1]
        arr = self._put(name, make_shards())
        self._committed[name] = (np.array(host_val, copy=True), arr)
        return arr

    def run(self, feeds):
        """feeds: dict name -> committed jax.Array (all in_names present)."""
        args = [feeds[n] for n in self.in_names]
        zeros = [np.zeros((NCORES * s[0],) + tuple(s[1:]), dt)
                 for s, dt in self.zero_shapes]
        outs = self.fn(*args, *zeros)
        return {n: np.asarray(outs[i]) for i, n in enumerate(self.out_names)}


_WMAP = {
    "w_dsd_21": "W_dsd_21", "w_dsd_22": "W_dsd_22",
    "w_dsd_11": "W_dsd_11", "w_dsd_12": "W_dsd_12",
    "w_usu_3": "W_usu_3", "w_usu_21": "W_usu_21",
    "w_usu_22": "W_usu_22", "w_usu_1": "W_usu_1",
}
_IXMAP = {
    "i_label": ("label", 1), "i_dsd1": ("dsd_1", 8), "i_dsd2": ("dsd_2", 64),
    "i_usu1": ("usu_1", 8), "i_usu2": ("usu_2", 64), "i_usu3": ("usu_3", 1024),
}


def kernel(**inputs):
    ex = _CACHE.get("ex")
    if ex is None:
        ex = _CACHE["ex"] = _Executor()

    feeds = {}
    for name, key, conv in (("Es", "E_s", np.float32), ("Ed", "E_d", np.float32)):
        raw = np.asarray(inputs[key])
        feeds[name] = ex.feed(
            name, raw,
            lambda raw=raw, conv=conv: [np.ascontiguousarray(raw, dtype=conv)] * NCORES)
    for name, key in _WMAP.items():
        raw = np.asarray(inputs[key])
        feeds[name] = ex.feed(
            name, raw,
            lambda raw=raw: [np.ascontiguousarray(
                np.asarray(raw, dtype=np.float32).T)] * NCORES)
    for name, (key, cols) in _IXMAP.items():
        raw = np.asarray(inputs[key])
        feeds[name] = ex.feed(
            name, raw,
            lambda raw=raw, cols=cols: [
                np.ascontiguousarray(
                    raw[c * BC:(c + 1) * BC].reshape(BC, cols).astype(np.int32))
                for c in range(NCORES)])

    outs = ex.run(feeds)
    return outs["score"].reshape(B).astype(np.float32)


# revision 6
# speedup vs baseline: 1.0726x; 1.0726x over previous
"""HGNN forward kernel for Trainium2, 8 NeuronCores, data-parallel over batch.

Device program (per core, batch chunk of 128) — unchanged from the working
baseline:
  - Embedding-row gathers via gpsimd indirect_dma_start, 128 rows/instruction
    (one row per partition, offsets [128,1] int32 in SBUF).
  - Neighbor-group sums computed by DMA-side accumulation (compute_op=add).
  - Math algebraically folded so every matmul is a 64x64 weight applied to
    transposed activations [64, N]; mean-over-neighbors pushed through the
    linear maps; avg_real weights computed on-device from the raw indices.

Execution path: custom PJRT shard_map executor (modeled on
bass2jax.run_bass_via_pjrt) with committed-device-array input caching.
Under axon, shipping the 8x-replicated embedding tables (~107MB) through
the tunnel dominates wall time; here every input is device_put once and
re-used on subsequent calls when its host content is unchanged (verified
by np.array_equal against a stored copy), so a steady-state call uploads
only the donated output buffer and runs the NEFF.
"""
import numpy as np

import concourse.bass as bass
import concourse.bacc as bacc
import concourse.mybir as mybir
import concourse.tile as tile
from concourse.masks import make_identity

F32 = mybir.dt.float32
I32 = mybir.dt.int32
AF = mybir.ActivationFunctionType
OP = mybir.AluOpType

NUM_SYMP, NUM_DISE = 50000, 2000
D = 64
B = 1024
NCORES = 8
BC = B // NCORES  # 128 batch elems per core

_CACHE = {}
_LAST_EXEC_NS = None


def _bcast_inner(ap, n):
    """Append a broadcast (step-0) innermost dim of size n to an AP."""
    return bass.AP(ap.tensor, ap.offset, list(ap.ap) + [[0, n]])


def _bcast_mid(ap, pos, n):
    """Insert a broadcast (step-0) dim of size n at position pos."""
    dims = list(ap.ap)
    return bass.AP(ap.tensor, ap.offset, dims[:pos] + [[0, n]] + dims[pos:])


def _build():
    nc = bacc.Bacc("TRN2", target_bir_lowering=False, debug=False)

    Es = nc.dram_tensor("Es", [NUM_SYMP + 1, D], F32, kind="ExternalInput")
    Ed = nc.dram_tensor("Ed", [NUM_DISE + 1, D], F32, kind="ExternalInput")
    wn = ["w_dsd_21", "w_dsd_22", "w_dsd_11", "w_dsd_12",
          "w_usu_3", "w_usu_21", "w_usu_22", "w_usu_1"]
    W = {n: nc.dram_tensor(n, [D, D], F32, kind="ExternalInput") for n in wn}
    i_label = nc.dram_tensor("i_label", [BC, 1], I32, kind="ExternalInput")
    i_dsd1 = nc.dram_tensor("i_dsd1", [BC, 8], I32, kind="ExternalInput")
    i_dsd2 = nc.dram_tensor("i_dsd2", [BC, 64], I32, kind="ExternalInput")
    i_usu1 = nc.dram_tensor("i_usu1", [BC, 8], I32, kind="ExternalInput")
    i_usu2 = nc.dram_tensor("i_usu2", [BC, 64], I32, kind="ExternalInput")
    i_usu3 = nc.dram_tensor("i_usu3", [BC, 1024], I32, kind="ExternalInput")
    out = nc.dram_tensor("score", [1, BC], F32, kind="ExternalOutput")

    with tile.TileContext(nc) as tc:
        with tc.tile_pool(name="const", bufs=1) as cst, \
             tc.tile_pool(name="main", bufs=1) as mp, \
             tc.tile_pool(name="ps", bufs=4, space="PSUM") as ps, \
             tc.tile_pool(name="psm", bufs=3, space="PSUM") as psm:

            ident = cst.tile([128, 128], F32)
            make_identity(nc, ident[:])
            ones1 = cst.tile([1, D], F32)
            nc.vector.memset(ones1[:], 1.0)
            ones64 = cst.tile([D, 1], F32)
            nc.vector.memset(ones64[:], 1.0)
            wt = {}
            for n in wn:
                wt[n] = cst.tile([D, D], F32, name=f"wt_{n}")
                nc.sync.dma_start(out=wt[n][:], in_=W[n][:])

            # ---- index tiles (single DMAs) ----
            ix_lab = mp.tile([BC, 1], I32)
            nc.sync.dma_start(out=ix_lab[:], in_=i_label[:])
            ix_d1 = mp.tile([BC, 8], I32)
            nc.sync.dma_start(out=ix_d1[:], in_=i_dsd1[:])
            ix_d2 = mp.tile([BC, 64], I32)
            nc.sync.dma_start(out=ix_d2[:], in_=i_dsd2[:])
            ix_u1 = mp.tile([BC, 8], I32)
            nc.sync.dma_start(out=ix_u1[:], in_=i_usu1[:])
            ix_u2 = mp.tile([BC, 64], I32)
            nc.sync.dma_start(out=ix_u2[:], in_=i_usu2[:])
            ix_u3 = mp.tile([BC, 1024], I32)
            nc.sync.dma_start(out=ix_u3[:], in_=i_usu3[:])

            def gather(dst_ap, table, off_ap, accum=False):
                nc.gpsimd.indirect_dma_start(
                    out=dst_ap, out_offset=None, in_=table[:],
                    in_offset=bass.IndirectOffsetOnAxis(ap=off_ap, axis=0),
                    compute_op=(OP.add if accum else OP.bypass),
                )


            def lrelu(dst_ap, src_ap, scratch_name):
                t = mp.tile(list(dst_ap.shape), F32, name=scratch_name, tag="lrt")
                nc.vector.tensor_scalar_mul(out=t[:], in0=src_ap, scalar1=0.2)
                nc.vector.tensor_tensor(out=dst_ap, in0=src_ap, in1=t[:], op=OP.max)

            # ---- plain gathers: td, es, u1 (rows used individually) ----
            td_std = mp.tile([BC, D], F32)
            gather(td_std[:], Ed, ix_lab[:, 0:1])
            es_std = mp.tile([BC, 8 * D], F32)
            u1_std = mp.tile([BC, 8 * D], F32)
            for h in range(8):
                gather(es_std[:, h * D:(h + 1) * D], Es, ix_d1[:, h:h + 1])
                gather(u1_std[:, h * D:(h + 1) * D], Es, ix_u1[:, h:h + 1])

            # ---- accumulating gathers: dsd_2 (8 nbrs), usu_3 (16 nbrs) ----
            acc_d2 = mp.tile([BC, 8 * D], F32)
            nc.vector.memset(acc_d2[:], 0.0)
            acc_u3 = mp.tile([BC, 64 * D], F32)
            nc.vector.memset(acc_u3[:], 0.0)
            for j in range(8):
                for m in range(8):
                    gather(acc_d2[:, m * D:(m + 1) * D], Ed,
                           ix_d2[:, m * 8 + j: m * 8 + j + 1], accum=True)
            for j in range(16):
                for m in range(64):
                    gather(acc_u3[:, m * D:(m + 1) * D], Es,
                           ix_u3[:, m * 16 + j: m * 16 + j + 1], accum=True)

            # ---- count weights w = (cnt>0) / (cnt + 1e-8) ----
            def count_w(ix_t, groups, j, name):
                f = mp.tile([BC, groups * j], F32, name=f"f_{name}")
                nc.vector.tensor_copy(out=f[:], in_=ix_t[:])
                z = mp.tile([BC, groups * j], F32, name=f"z_{name}")
                nc.vector.tensor_scalar(out=z[:], in0=f[:], scalar1=0.0,
                                        scalar2=None, op0=OP.is_equal)
                zc = mp.tile([BC, groups], F32, name=f"zc_{name}")
                nc.vector.tensor_reduce(
                    out=zc[:],
                    in_=z[:].rearrange("p (g j) -> p g j", g=groups, j=j),
                    axis=mybir.AxisListType.X, op=OP.add)
                cnt = mp.tile([BC, groups], F32, name=f"cnt_{name}")
                nc.vector.tensor_scalar(out=cnt[:], in0=zc[:], scalar1=-1.0,
                                        scalar2=float(j), op0=OP.mult, op1=OP.add)
                mpos = mp.tile([BC, groups], F32, name=f"mp_{name}")
                nc.vector.tensor_scalar(out=mpos[:], in0=cnt[:], scalar1=1.0,
                                        scalar2=None, op0=OP.min)
                ce = mp.tile([BC, groups], F32, name=f"ce_{name}")
                nc.vector.tensor_scalar(out=ce[:], in0=cnt[:], scalar1=1e-8,
                                        scalar2=None, op0=OP.add)
                r = mp.tile([BC, groups], F32, name=f"r_{name}")
                nc.vector.reciprocal(out=r[:], in_=ce[:])
                w = mp.tile([BC, groups], F32, name=f"w_{name}")
                nc.vector.tensor_tensor(out=w[:], in0=r[:], in1=mpos[:], op=OP.mult)
                return w

            w_d2 = count_w(ix_d2, 8, 8, "d2")     # [128, 8]
            w_u3 = count_w(ix_u3, 64, 16, "u3")   # [128, 64]
            w_u2 = count_w(ix_u2, 8, 8, "u2")     # [128, 8]
            w_d1 = count_w(ix_d1, 1, 8, "d1")     # [128, 1]
            w_u1 = count_w(ix_u1, 1, 8, "u1")     # [128, 1]

            # ---- scale accumulated sums by group weights (std layout) ----
            nc.vector.tensor_tensor(
                out=acc_d2[:].rearrange("p (m d) -> p m d", m=8, d=D),
                in0=acc_d2[:].rearrange("p (m d) -> p m d", m=8, d=D),
                in1=_bcast_inner(w_d2[:], D), op=OP.mult)
            nc.vector.tensor_tensor(
                out=acc_u3[:].rearrange("p (m d) -> p m d", m=64, d=D),
                in0=acc_u3[:].rearrange("p (m d) -> p m d", m=64, d=D),
                in1=_bcast_inner(w_u3[:], D), op=OP.mult)

            # ---- transposes into [64, cols] matmul layout ----
            def transpose_into(dstT, src_std, nblk):
                for m in range(nblk):
                    p = ps.tile([D, 128], F32, name="tp", tag="tp")
                    nc.tensor.transpose(out=p[:], in_=src_std[:, m * D:(m + 1) * D],
                                        identity=ident[:])
                    nc.vector.tensor_copy(out=dstT[:, m * 128:(m + 1) * 128], in_=p[:])

            tdT = mp.tile([D, 128], F32)
            transpose_into(tdT, td_std, 1)
            esT = mp.tile([D, 8 * 128], F32)
            transpose_into(esT, es_std, 8)
            u1T = mp.tile([D, 8 * 128], F32)
            transpose_into(u1T, u1_std, 8)
            edmT = mp.tile([D, 8 * 128], F32)
            transpose_into(edmT, acc_d2, 8)
            s3T = mp.tile([D, 64 * 128], F32)
            transpose_into(s3T, acc_u3, 64)

            # ---- replicated column weights via transpose + K=1 matmul ----
            def replicate_cols(w_t, groups, name):
                rep = mp.tile([D, groups * 128], F32, name=f"rep_{name}")
                for g in range(groups):
                    pt = ps.tile([2, 128], F32, name="wtp", tag="tp")
                    nc.tensor.transpose(out=pt[0:1, :], in_=w_t[:, g:g + 1],
                                        identity=ident[:])
                    wg = mp.tile([1, 128], F32, name=f"wg_{name}")
                    nc.vector.tensor_copy(out=wg[:], in_=pt[0:1, :])
                    pr = ps.tile([D, 128], F32, name="wrep", tag="tp")
                    nc.tensor.matmul(out=pr[:], lhsT=ones1[:], rhs=wg[:],
                                     start=True, stop=True)
                    nc.vector.tensor_copy(out=rep[:, g * 128:(g + 1) * 128], in_=pr[:])
                return rep

            w2u_rep = replicate_cols(w_u2, 8, "u2")    # [64, 1024]
            w1u_rep = replicate_cols(w_u1, 1, "u1")    # [64, 128]
            w1d_rep = replicate_cols(w_d1, 1, "d1")    # [64, 128]

            # ---- usu path ----
            # eu2 = lrelu(W3 @ (w3 * sum_j s3)) ; cols (m=u1*8+u2, b)
            eu2T = mp.tile([D, 64 * 128], F32)
            for ch in range(16):
                pm = psm.tile([D, 512], F32, name="mm3", tag="mm")
                nc.tensor.matmul(out=pm[:], lhsT=wt["w_usu_3"][:],
                                 rhs=s3T[:, ch * 512:(ch + 1) * 512],
                                 start=True, stop=True)
                lrelu(eu2T[:, ch * 512:(ch + 1) * 512], pm[:], "lr3")

            # su1 = sum_u2 eu2 ; su2 = sum_u2 (eu2 * u1)  -> cols (u1, b)
            su1 = mp.tile([D, 8 * 128], F32)
            ev = eu2T[:].rearrange("p (u v b) -> p u b v", u=8, v=8, b=128)
            nc.vector.tensor_reduce(
                out=su1[:].rearrange("p (u b) -> p u b", u=8, b=128),
                in_=ev, axis=mybir.AxisListType.X, op=OP.add)
            tmp = mp.tile([D, 64 * 128], F32)
            u1bc = _bcast_mid(u1T[:].rearrange("p (u b) -> p u b", u=8, b=128), 2, 8)
            nc.vector.tensor_tensor(
                out=tmp[:].rearrange("p (u v b) -> p u v b", u=8, v=8, b=128),
                in0=eu2T[:].rearrange("p (u v b) -> p u v b", u=8, v=8, b=128),
                in1=u1bc, op=OP.mult)
            su2 = mp.tile([D, 8 * 128], F32)
            nc.vector.tensor_reduce(
                out=su2[:].rearrange("p (u b) -> p u b", u=8, b=128),
                in_=tmp[:].rearrange("p (u v b) -> p u b v", u=8, v=8, b=128),
                axis=mybir.AxisListType.X, op=OP.add)

            # rhs1 = su1*w2 + u1T ; rhs2 = su2*w2
            rhs1 = mp.tile([D, 8 * 128], F32)
            nc.vector.tensor_tensor(out=rhs1[:], in0=su1[:], in1=w2u_rep[:], op=OP.mult)
            nc.vector.tensor_tensor(out=rhs1[:], in0=rhs1[:], in1=u1T[:], op=OP.add)
            rhs2 = mp.tile([D, 8 * 128], F32)
            nc.vector.tensor_tensor(out=rhs2[:], in0=su2[:], in1=w2u_rep[:], op=OP.mult)

            es1 = mp.tile([D, 8 * 128], F32)
            for ch in range(2):
                sl = slice(ch * 512, (ch + 1) * 512)
                pm = psm.tile([D, 512], F32, name="mmu", tag="mm")
                nc.tensor.matmul(out=pm[:], lhsT=wt["w_usu_21"][:], rhs=rhs1[:, sl],
                                 start=True, stop=False)
                nc.tensor.matmul(out=pm[:], lhsT=wt["w_usu_22"][:], rhs=rhs2[:, sl],
                                 start=False, stop=True)
                lrelu(es1[:, sl], pm[:], "lru")

            # emb_user = lrelu(W1u @ (w1u * sum_u1 es1))
            rU = mp.tile([D, 128], F32)
            nc.vector.tensor_reduce(
                out=rU[:],
                in_=es1[:].rearrange("p (u b) -> p b u", u=8, b=128),
                axis=mybir.AxisListType.X, op=OP.add)
            nc.vector.tensor_tensor(out=rU[:], in0=rU[:], in1=w1u_rep[:], op=OP.mult)
            pmU = ps.tile([D, 128], F32, name="mmU", tag="tp")
            nc.tensor.matmul(out=pmU[:], lhsT=wt["w_usu_1"][:], rhs=rU[:],
                             start=True, stop=True)
            embU = mp.tile([D, 128], F32)
            lrelu(embU[:], pmU[:], "lrU")

            # ---- dsd path ----
            rhsA = mp.tile([D, 8 * 128], F32)
            nc.vector.tensor_tensor(out=rhsA[:], in0=edmT[:], in1=esT[:], op=OP.add)
            rhsB = mp.tile([D, 8 * 128], F32)
            nc.vector.tensor_tensor(out=rhsB[:], in0=edmT[:], in1=esT[:], op=OP.mult)
            es1d = mp.tile([D, 8 * 128], F32)
            for ch in range(2):
                sl = slice(ch * 512, (ch + 1) * 512)
                pm = psm.tile([D, 512], F32, name="mmd", tag="mm")
                nc.tensor.matmul(out=pm[:], lhsT=wt["w_dsd_21"][:], rhs=rhsA[:, sl],
                                 start=True, stop=False)
                nc.tensor.matmul(out=pm[:], lhsT=wt["w_dsd_22"][:], rhs=rhsB[:, sl],
                                 start=False, stop=True)
                lrelu(es1d[:, sl], pm[:], "lrd")

            r1 = mp.tile([D, 128], F32)
            nc.vector.tensor_reduce(
                out=r1[:],
                in_=es1d[:].rearrange("p (h b) -> p b h", h=8, b=128),
                axis=mybir.AxisListType.X, op=OP.add)
            tmp2 = mp.tile([D, 8 * 128], F32)
            tdbc = _bcast_mid(tdT[:], 1, 8)
            nc.vector.tensor_tensor(
                out=tmp2[:].rearrange("p (h b) -> p h b", h=8, b=128),
                in0=es1d[:].rearrange("p (h b) -> p h b", h=8, b=128),
                in1=tdbc, op=OP.mult)
            r2 = mp.tile([D, 128], F32)
            nc.vector.tensor_reduce(
                out=r2[:],
                in_=tmp2[:].rearrange("p (h b) -> p b h", h=8, b=128),
                axis=mybir.AxisListType.X, op=OP.add)
            m1 = mp.tile([D, 128], F32)
            nc.vector.tensor_tensor(out=m1[:], in0=r1[:], in1=w1d_rep[:], op=OP.mult)
            nc.vector.tensor_tensor(out=m1[:], in0=m1[:], in1=tdT[:], op=OP.add)
            m2 = mp.tile([D, 128], F32)
            nc.vector.tensor_tensor(out=m2[:], in0=r2[:], in1=w1d_rep[:], op=OP.mult)
            pmD = ps.tile([D, 128], F32, name="mmD", tag="tp")
            nc.tensor.matmul(out=pmD[:], lhsT=wt["w_dsd_11"][:], rhs=m1[:],
                             start=True, stop=False)
            nc.tensor.matmul(out=pmD[:], lhsT=wt["w_dsd_12"][:], rhs=m2[:],
                             start=False, stop=True)
            embD = mp.tile([D, 128], F32)
            lrelu(embD[:], pmD[:], "lrD")

            # ---- score ----
            prod = mp.tile([D, 128], F32)
            nc.vector.tensor_tensor(out=prod[:], in0=embD[:], in1=embU[:], op=OP.mult)
            pS = ps.tile([2, 128], F32, name="mmS", tag="tp")
            nc.tensor.matmul(out=pS[0:1, :], lhsT=ones64[:], rhs=prod[:],
                             start=True, stop=True)
            score_sb = mp.tile([1, 128], F32)
            nc.vector.tensor_copy(out=score_sb[:], in_=pS[0:1, :])
            nc.sync.dma_start(out=out[:], in_=score_sb[:])

    nc.finalize()
    return nc


class _Executor:
    """shard_map/PJRT executor with committed-device-array input caching.

    Per BIR input we keep (host_copy, committed_global_array). On each call
    the freshly prepped host value is compared against host_copy; on match
    the committed jax.Array (already resident on the 8 cores with the right
    sharding) is passed to jit directly, so no bytes cross the axon tunnel.
    """

    def __init__(self):
        import jax
        from jax.sharding import Mesh, PartitionSpec, NamedSharding
        from jax.experimental.shard_map import shard_map
        from concourse import bass2jax

        self.jax = jax
        bass2jax.install_neuronx_cc_hook()
        nc = _build()
        assert nc.dbg_addr is None
        self.nc = nc
        partition_name = (nc.partition_id_tensor.name
                          if nc.partition_id_tensor else None)

        in_names, out_names, out_avals, zero_shapes = [], [], [], []
        for alloc in nc.m.functions[0].allocations:
            if not isinstance(alloc, mybir.MemoryLocationSet):
                continue
            assert alloc.memorylocations
            name = alloc.memorylocations[0].name
            if alloc.kind == "ExternalInput":
                if name != partition_name:
                    in_names.append(name)
            elif alloc.kind == "ExternalOutput":
                shape = tuple(alloc.tensor_shape)
                dtype = mybir.dt.np(alloc.dtype)
                out_names.append(name)
                out_avals.append(jax.core.ShapedArray(shape, dtype))
                zero_shapes.append((shape, dtype))
        self.in_names = in_names
        self.out_names = out_names
        self.out_avals = out_avals
        self.zero_shapes = zero_shapes
        n_params = len(in_names)
        n_outs = len(out_names)

        devices = jax.devices()[:NCORES]
        assert len(devices) == NCORES
        self.devices = devices
        self.mesh = Mesh(np.asarray(devices), ("core",))
        self.sharding = NamedSharding(self.mesh, PartitionSpec("core"))

        all_names = tuple(in_names) + tuple(out_names)
        if partition_name is not None:
            all_names = all_names + (partition_name,)

        def _body(*args):
            operands = list(args)
            if partition_name is not None:
                operands.append(bass2jax.partition_id_tensor())
            outs = bass2jax._bass_exec_p.bind(
                *operands,
                out_avals=tuple(out_avals),
                in_names=all_names,
                out_names=tuple(out_names),
                lowering_input_output_aliases=(),
                sim_require_finite=True,
                sim_require_nnan=True,
                nc=nc,
            )
            return tuple(outs)

        donate = tuple(range(n_params, n_params + n_outs))
        self.fn = jax.jit(
            shard_map(_body, mesh=self.mesh,
                      in_specs=(PartitionSpec("core"),) * (n_params + n_outs),
                      out_specs=(PartitionSpec("core"),) * n_outs,
                      check_rep=False),
            donate_argnums=donate, keep_unused=True)

        # name -> (host_copy, committed jax.Array)
        self._committed = {}

    def _put(self, name, shards):
        """Commit per-core host shards as one global array on the 8 cores."""
        jax = self.jax
        dev_arrs = [jax.device_put(a, d) for a, d in zip(shards, self.devices)]
        gshape = (NCORES * shards[0].shape[0],) + tuple(shards[0].shape[1:])
        return jax.make_array_from_single_device_arrays(
            gshape, self.sharding, dev_arrs)

    def feed(self, name, host_val, make_shards):
        """Return (committed array, hit) for `name`; re-upload iff changed.

        host_val: cheap-to-compare host array identifying the content.
        make_shards: () -> list of NCORES per-core np arrays (called lazily,
        only on miss).
        """
        ent = self._committed.get(name)
        if ent is not None and host_val.shape == ent[0].shape \
                and host_val.dtype == ent[0].dtype \
                and np.array_equal(host_val, ent[0]):
            return ent[1], True
        arr = self._put(name, make_shards())
        self._committed[name] = (np.array(host_val, copy=True), arr)
        return arr, False

    def cached_args(self):
        """All committed arrays in in_names order, or None if any missing."""
        try:
            return [self._committed[n][1] for n in self.in_names]
        except KeyError:
            return None

    def dispatch(self, args):
        zeros = [np.zeros((NCORES * s[0],) + tuple(s[1:]), dt)
                 for s, dt in self.zero_shapes]
        return self.fn(*args, *zeros)


_WMAP = {
    "w_dsd_21": "W_dsd_21", "w_dsd_22": "W_dsd_22",
    "w_dsd_11": "W_dsd_11", "w_dsd_12": "W_dsd_12",
    "w_usu_3": "W_usu_3", "w_usu_21": "W_usu_21",
    "w_usu_22": "W_usu_22", "w_usu_1": "W_usu_1",
}
_IXMAP = {
    "i_label": ("label", 1), "i_dsd1": ("dsd_1", 8), "i_dsd2": ("dsd_2", 64),
    "i_usu1": ("usu_1", 8), "i_usu2": ("usu_2", 64), "i_usu3": ("usu_3", 1024),
}


def kernel(**inputs):
    import threading

    ex = _CACHE.get("ex")
    if ex is None:
        ex = _CACHE["ex"] = _Executor()

    # Optimistic dispatch: if every input has a committed device copy from a
    # prior call, launch the NEFF on those immediately and start pulling the
    # result in a background thread; the host-side content verification below
    # then runs inside the ~80ms tunnel-roundtrip window. The optimistic
    # result is only used if verification confirms nothing changed.
    opt_args = ex.cached_args()
    opt_box = []
    opt_thread = None
    if opt_args is not None:
        opt_outs = ex.dispatch(opt_args)

        def _fetch():
            try:
                opt_box.append(np.asarray(opt_outs[0]))
            except BaseException as e:  # surface fetch errors to caller
                opt_box.append(e)

        opt_thread = threading.Thread(target=_fetch)
        opt_thread.start()

    feeds = {}
    all_hit = True
    for name, key, conv in (("Es", "E_s", np.float32), ("Ed", "E_d", np.float32)):
        raw = np.asarray(inputs[key])
        feeds[name], hit = ex.feed(
            name, raw,
            lambda raw=raw, conv=conv: [np.ascontiguousarray(raw, dtype=conv)] * NCORES)
        all_hit &= hit
    for name, key in _WMAP.items():
        raw = np.asarray(inputs[key])
        feeds[name], hit = ex.feed(
            name, raw,
            lambda raw=raw: [np.ascontiguousarray(
                np.asarray(raw, dtype=np.float32).T)] * NCORES)
        all_hit &= hit
    for name, (key, cols) in _IXMAP.items():
        raw = np.asarray(inputs[key])
        feeds[name], hit = ex.feed(
            name, raw,
            lambda raw=raw, cols=cols: [
                np.ascontiguousarray(
                    raw[c * BC:(c + 1) * BC].reshape(BC, cols).astype(np.int32))
                for c in range(NCORES)])
        all_hit &= hit

    if opt_thread is not None and all_hit:
        opt_thread.join()
        res = opt_box[0]
        if isinstance(res, BaseException):
            raise res
        score = res
    else:
        outs = ex.dispatch([feeds[n] for n in ex.in_names])
        score = np.asarray(outs[0])
    return score.reshape(B).astype(np.float32)
